# revision 1
# baseline (speedup 1.0000x reference)
"""Trainium2 Bass kernel for nn_AutocorrelationCorrelogram.

For nervegram [B=4, F=50, T=20000, C=2]: 300 periodic-Hann-windowed frames
of length 512 per (b,f,c) signal, circular autocorrelation via
Wiener-Khinchin (rfft -> |.|^2 -> irfft), relu, normalize by sqrt(zero
lag), keep 256 lags, mean over channels -> [4, 50, 300, 256].

Sharding: pure data parallel over the 200 (b,f) pairs -> 25 per core x 8
cores (SPMD, no collectives).

v3 design (bf16, host-side frame/transpose prep, PE-saturating schedule):
  - host pre-frames and pre-transposes the signal into the time-major
    moving-operand layout yt[sb, c, t(128), k(4), row(500)] in bf16, so
    the device does zero data-movement work on the PE: one 512KB DMA per
    (sb, c) lands the rfft moving operand directly
  - rfft as bf16 matmuls with the Hann window folded into the DFT
    matrices; wsin col 0 carries the bin-256 cos column
  - P = Re^2 + Im^2: ACT Square (bf16 out), ph add split DVE/GpSimd
  - irfft uses P as the stationary operand -> acf^T [125 rows, 4 groups,
    256 lags] in one 2-bank PSUM tile; D scaled by 0.25 so adding the
    two channels yields the channel mean of the normalized acf
  - norm: one batched ACT Sqrt + DVE reciprocal over the 4 zero-lag
    columns, then relu(acf*rcc) fused into a single
    scalar_tensor_tensor (mult, max-with-0) per group
  - PE stream is software-pipelined: irfft of superbatch N-1 is emitted
    between the rfft phases of superbatch N so the tensor engine never
    waits on the elementwise chain
"""

import sys

import numpy as np

sys.path.insert(0, "/opt/trn_rl_repo")

B, F, T, C = 4, 50, 20000, 2
NUM_FRAME = 300
LEN_FRAME = 512
LAGS = 256
NBINS = 257
N_CORES = 8
BF_PER_CORE = (B * F) // N_CORES  # 25

FRAMES_PER_SB = 20
TILES_PER_SB = 4
N_SB = NUM_FRAME // FRAMES_PER_SB  # 15
NCOLS = 500  # (20 frames x 25 bf) per channel

STARTS = np.linspace(0, T - LEN_FRAME, NUM_FRAME).astype(np.int64)


def build_weights():
    t = np.arange(LEN_FRAME, dtype=np.float64)
    w = 0.5 - 0.5 * np.cos(2.0 * np.pi * t / LEN_FRAME)  # periodic hann
    ang = 2.0 * np.pi * np.outer(t, np.arange(NBINS)) / LEN_FRAME
    Cm = np.cos(ang) * w[:, None]  # [512, 257]
    Sm = -np.sin(ang) * w[:, None]
    wcos = Cm[:, 0:256].reshape(4, 128, 256).copy()
    wsin = Sm[:, 0:256].reshape(4, 128, 256).copy()
    wsin[:, :, 0] = Cm[:, 256].reshape(4, 128)  # bin-256 cos column
    alpha = 0.25  # folds the channel-mean 0.5 (output scales with sqrt(alpha))
    k = np.arange(NBINS)
    coef = np.full(NBINS, 2.0)
    coef[0] = 1.0
    coef[256] = 1.0
    D = (alpha / LEN_FRAME) * coef[:, None] * np.cos(
        2.0 * np.pi * np.outer(k, np.arange(LAGS)) / LEN_FRAME
    )
    # ph(h0) row 0 carries P[0]+P[256] (sq_i row 0 is P[256] via the wsin
    # col-0 trick and is not masked); compensate exactly in the bin-256 row
    D[256] -= D[0]
    return wcos, wsin, D


def build_nc(n_sb=N_SB):
    from contextlib import ExitStack

    import concourse.bacc as bacc
    import concourse.tile as tile
    from concourse import mybir

    f32 = mybir.dt.float32
    bf16 = mybir.dt.bfloat16
    AF = mybir.ActivationFunctionType
    ALU = mybir.AluOpType

    nc = bacc.Bacc("TRN2", target_bir_lowering=False, debug=False)

    yt_d = nc.dram_tensor(
        "yt", [N_SB, C, 128, TILES_PER_SB, NCOLS], bf16, kind="ExternalInput"
    ).ap()
    wcos_d = nc.dram_tensor("wcos", [4, 128, 256], bf16, kind="ExternalInput").ap()
    wsin_d = nc.dram_tensor("wsin", [4, 128, 256], bf16, kind="ExternalInput").ap()
    dmat_d = nc.dram_tensor("dmat", [NBINS, LAGS], bf16, kind="ExternalInput").ap()
    out = nc.dram_tensor(
        "out", [BF_PER_CORE, NUM_FRAME, LAGS], f32, kind="ExternalOutput"
    ).ap()

    with tile.TileContext(nc) as tc, ExitStack() as ctx:
        consts = ctx.enter_context(tc.tile_pool(name="consts", bufs=1))
        sbp = ctx.enter_context(tc.tile_pool(name="work", bufs=1))
        pp = ctx.enter_context(tc.tile_pool(name="ps", bufs=1, space="PSUM"))

        yt_q = {}  # (s, c) -> yt tile [128, 4, 500]

        def load_yt(s, split=False):
            # c0 on gpsimd, c1 on sync: never issue DMAs from the scalar
            # queue (it shares the ACT engine with the squares)
            for c in range(C):
                eng = nc.gpsimd if c == 0 else nc.sync
                if split:
                    # per-k tiles: the first matmul waits on one 128KB DMA
                    # instead of the full 512KB load
                    ts = []
                    for k in range(TILES_PER_SB):
                        tk = sbp.tile(
                            [128, NCOLS], bf16, tag="yt0", bufs=16,
                            name=f"yt0_{c}_{k}",
                        )
                        eng.dma_start(out=tk[:], in_=yt_d[s, c, :, k, :])
                        ts.append(tk)
                    yt_q[(s, c)] = ts
                else:
                    t = sbp.tile([128, TILES_PER_SB, NCOLS], bf16, tag="yt", bufs=8)
                    eng.dma_start(out=t[:], in_=yt_d[s, c])
                    yt_q[(s, c)] = t

        # issue the first moving-operand loads before the const DMAs so the
        # first rfft matmuls aren't serialized behind them
        load_yt(0, split=True)

        # ---- constants (per-k tiles, interleaved across two queues in
        # first-use order so the first R phase's k-loop never outruns them) ----
        wcos_sb = [
            consts.tile([128, 256], bf16, tag=f"wcos{k}", name=f"wcos_sb{k}")
            for k in range(4)
        ]
        wsin_sb = [
            consts.tile([128, 256], bf16, tag=f"wsin{k}", name=f"wsin_sb{k}")
            for k in range(4)
        ]
        for k in range(4):
            eng = nc.sync if k % 2 == 0 else nc.scalar
            eng.dma_start(out=wcos_sb[k][:], in_=wcos_d[k])
        for k in range(4):
            eng = nc.sync if k % 2 == 0 else nc.scalar
            eng.dma_start(out=wsin_sb[k][:], in_=wsin_d[k])
        dm0 = consts.tile([128, 256], bf16, tag="dm0")
        dm1 = consts.tile([128, 256], bf16, tag="dm1")
        nc.scalar.dma_start(out=dm0[:], in_=dmat_d[0:128])
        nc.scalar.dma_start(out=dm1[:], in_=dmat_d[128:256])
        zero_b = consts.tile([128, 1], f32, tag="zerob")
        nc.vector.memset(zero_b[:], 0.0)
        eps_b = consts.tile([128, 1], f32, tag="epsb")
        nc.vector.memset(eps_b[:], 1e-30)
        zeros_l = consts.tile([128, LAGS], f32, tag="zerosl")
        nc.vector.memset(zeros_l[:], 0.0)
        # dm2 padded to a full 128-row moving operand (rows 1..127 zero) so
        # the bin-256 matmul can use the full sq_i tile as stationary
        dm2z = consts.tile([128, 256], bf16, tag="dm2z")
        nc.vector.memset(dm2z[:], 0.0)
        nc.sync.dma_start(out=dm2z[0:1, :], in_=dmat_d[256:257])

        ph_q = {}  # (s, c, h) -> ph tile bf16 [128, 500]
        p256_q = {}  # (s, c) -> sq_i(h0) tile (row 0 is P[256])

        def R_phase(s, c, h):
            rp = pp.tile([128, NCOLS], f32, tag="fft", bufs=4)
            ip = pp.tile([128, NCOLS], f32, tag="fft", bufs=4)
            yt = yt_q[(s, c)]
            def mov(k):
                return yt[k][:] if isinstance(yt, list) else yt[:, k, :]

            for k in range(4):
                nc.tensor.matmul(
                    rp[:],
                    wcos_sb[k][:, 128 * h : 128 * h + 128],
                    mov(k),
                    start=(k == 0),
                    stop=(k == 3),
                )
            for k in range(4):
                nc.tensor.matmul(
                    ip[:],
                    wsin_sb[k][:, 128 * h : 128 * h + 128],
                    mov(k),
                    start=(k == 0),
                    stop=(k == 3),
                )
            sq_r = sbp.tile([128, NCOLS], bf16, tag="sqr", bufs=6)
            nc.scalar.activation(sq_r[:], rp[:], AF.Square, bias=zero_b[:])
            ph = sbp.tile([128, NCOLS], bf16, tag="ph", bufs=10)
            sq_i = sbp.tile([128, NCOLS], bf16, tag="sqi", bufs=6)
            nc.scalar.activation(sq_i[:], ip[:], AF.Square, bias=zero_b[:])
            if h == 0:
                # sq_i row 0 = Im_h0[0]^2 = P[256] (wsin col 0 carries
                # cos-256). The bin-256 matmul uses the FULL sq_i tile as
                # stationary; dm2z's zero rows 1..127 nullify the other
                # contraction terms. ph row 0 = P[0]+P[256]; dmat row 256 is
                # D[256]-D[0] to compensate exactly.
                nc.vector.tensor_add(ph[:], sq_r[:], sq_i[:])
                p256_q[(s, c)] = sq_i
            else:
                nc.vector.tensor_add(ph[:], sq_r[:], sq_i[:])
            ph_q[(s, c, h)] = ph

        def I_phase(s1, c):
            """irfft matmuls for one channel; norm chain is emitted later."""
            ph0, ph1 = ph_q[(s1, c, 0)], ph_q[(s1, c, 1)]
            p256 = p256_q[(s1, c)]
            acfp = pp.tile([125, 4, LAGS], f32, tag="acf", bufs=2)
            for g in range(4):
                sl = slice(125 * g, 125 * g + 125)
                nc.tensor.matmul(
                    acfp[:, g, :], ph0[:, sl], dm0[:], start=True, stop=False
                )
                nc.tensor.matmul(
                    acfp[:, g, :], ph1[:, sl], dm1[:], start=False, stop=False
                )
                nc.tensor.matmul(
                    acfp[:, g, :], p256[:, sl], dm2z[:], start=False, stop=True
                )
            return acfp

        def norm_phase(c, acfp, split=False):
            """sqrt -> recip -> fused relu-scale; emitted after all squares.

            split=True runs half the groups as ACT Relu+scale so the drain's
            norm chain parallelizes across ACT and DVE."""
            sqc = sbp.tile([125, 4], f32, tag="sqc", bufs=6)
            nc.scalar.activation(sqc[:], acfp[:, :, 0], AF.Sqrt, bias=eps_b[:125])
            rcc = sbp.tile([125, 4], f32, tag="rcc", bufs=6)
            nc.vector.reciprocal(out=rcc[:], in_=sqc[:])
            nts = []
            for g in range(4):
                nt = sbp.tile([125, LAGS], f32, tag=f"nt{c}", bufs=6)
                if split and g >= 2:
                    nc.scalar.activation(
                        nt[:],
                        acfp[:, g, :],
                        AF.Relu,
                        bias=zero_b[:125],
                        scale=rcc[:, g : g + 1],
                    )
                else:
                    nc.vector.scalar_tensor_tensor(
                        out=nt[:],
                        in0=acfp[:, g, :],
                        scalar=rcc[:, g : g + 1],
                        in1=zeros_l[:125, :],
                        op0=ALU.mult,
                        op1=ALU.max,
                    )
                nts.append(nt)
            return nts

        # ---- pipeline ----
        load_yt(1)

        def store_sb(s1, nts_c0, nts_c1, final=False):
            mt = sbp.tile([125, 4, LAGS], f32, tag="mt", bufs=3)
            m0 = s1 * FRAMES_PER_SB
            for g in range(4):
                eng_add = nc.gpsimd if g % 2 == 0 else nc.vector
                eng_add.tensor_add(mt[:, g, :], nts_c0[g][:], nts_c1[g][:])
                mf = m0 + 5 * g
                eng = nc.sync if (not final or g % 2 == 0) else nc.gpsimd
                eng.dma_start(
                    out=out[:, mf : mf + 5, :].rearrange("bf mm l -> mm bf l"),
                    in_=mt[:, g, :],
                )
            for c in range(C):
                for h in range(2):
                    ph_q.pop((s1, c, h), None)
                p256_q.pop((s1, c), None)
                yt_q.pop((s1, c), None)

        for it in range(n_sb):
            s1 = it - 1
            last = it == n_sb - 1
            if it + 2 < n_sb:
                load_yt(it + 2)

            acf_c0 = acf_c1 = None
            if s1 >= 0:
                acf_c0 = I_phase(s1, 0)
            R_phase(it, 0, 0)
            R_phase(it, 0, 1)
            if s1 >= 0:
                acf_c1 = I_phase(s1, 1)
            if not last:
                R_phase(it, 1, 0)
                if s1 >= 0:
                    nts_c0 = norm_phase(0, acf_c0)
                R_phase(it, 1, 1)
                if s1 >= 0:
                    nts_c1 = norm_phase(1, acf_c1)
                    store_sb(s1, nts_c0, nts_c1)
            else:
                # drain: overlap the final superbatch's irfft/norm with the
                # last R phases so the tail chain is short
                nts_c0 = norm_phase(0, acf_c0)  # frees acf buf for I(it, 0)
                R_phase(it, 1, 0)
                acf_l0 = I_phase(it, 0)
                R_phase(it, 1, 1)
                nts_c1 = norm_phase(1, acf_c1)
                store_sb(s1, nts_c0, nts_c1)
                acf_l1 = I_phase(it, 1)
                nts_l0 = norm_phase(0, acf_l0, split=True)
                nts_l1 = norm_phase(1, acf_l1, split=True)
                store_sb(it, nts_l0, nts_l1, final=True)

    nc.compile()
    return nc


_NC_CACHE = {}


def _get_nc():
    if "nc" not in _NC_CACHE:
        _NC_CACHE["nc"] = build_nc()
    return _NC_CACHE["nc"]


def make_in_maps(nerv):
    import ml_dtypes

    bf16 = ml_dtypes.bfloat16
    xs = nerv.reshape(B * F, T, C)
    idx = STARTS[:, None] + np.arange(LEN_FRAME)  # [300, 512]
    wcos, wsin, dmat = build_weights()
    wcos = wcos.astype(bf16)
    wsin = wsin.astype(bf16)
    dmat = dmat.astype(bf16)
    maps = []
    for i in range(N_CORES):
        xc = xs[BF_PER_CORE * i : BF_PER_CORE * (i + 1)]  # [25, T, 2]
        fr = xc[:, idx, :].astype(bf16)  # [25, 300, 512, 2]
        # -> [sb, c, t, k, m_local, bf]
        yt = fr.reshape(BF_PER_CORE, N_SB, FRAMES_PER_SB, 4, 128, C).transpose(
            1, 5, 4, 3, 2, 0
        )
        yt = np.ascontiguousarray(yt).reshape(N_SB, C, 128, 4, NCOLS)
        maps.append({"yt": yt, "wcos": wcos, "wsin": wsin, "dmat": dmat})
    return maps


def kernel(nervegram, trace=False, **_ignored):
    from concourse.bass_utils import run_bass_kernel_spmd

    nerv = np.ascontiguousarray(np.asarray(nervegram, dtype=np.float32))
    assert nerv.shape == (B, F, T, C)
    in_maps = make_in_maps(nerv)
    nc = _get_nc()
    res = run_bass_kernel_spmd(nc, in_maps, list(range(N_CORES)), trace=trace)
    full = np.concatenate([res.results[i]["out"] for i in range(N_CORES)], axis=0)
    out = full.reshape(B, F, NUM_FRAME, LAGS)
    if trace:
        return out, res
    return out



# revision 7
# speedup vs baseline: 1.0786x; 1.0786x over previous
"""Trainium2 Bass kernel for nn_AutocorrelationCorrelogram.

For nervegram [B=4, F=50, T=20000, C=2]: 300 periodic-Hann-windowed frames
of length 512 per (b,f,c) signal, circular autocorrelation via
Wiener-Khinchin (rfft -> |.|^2 -> irfft), relu, normalize by sqrt(zero
lag), keep 256 lags, mean over channels -> [4, 50, 300, 256].

Sharding: pure data parallel over the 200 (b,f) pairs -> 25 per core x 8
cores (SPMD, no collectives).

v4 design (radix-2 DIF rfft, halves PE streaming vs dense DFT):
  - host pre-frames the signal into k-tiles y[128k+n] (time-major bf16,
    same yt layout as v3): one 512KB DMA per (sb, c)
  - front-end on DVE (bf16 2x/4x): window w folded in via
    tensor_scalar + scalar_tensor_tensor butterflies
      a = w[:256]*yL + w[256:]*yH,  d = w[:256]*yL - w[256:]*yH
  - rfft stage-2 on PE: even bins X[2b] = DFT_256(a), odd bins
    X[2b+1] = sum_n d[n] w512^{(2b+1)n}; 4 PSUM tiles [128x500]
    (er / ei(row0=bin-256-re) / or / oi), 8 matmuls x 500 cols per
    (sb, c) -- half the dense cost
  - squares on ACT (PSUM f32 -> SBUF bf16), no pair-adds: the irfft
    contracts all 4 square tiles directly; the D-row mapping makes
    bin 256 exact for free (D_ei row 0 = D-row-256)
  - irfft: stationary = 128-wide square-tile slices (FWL-eligible),
    moving = D_er/D_ei/D_o [128x256] bf16; acfp [128,4,256] f32 PSUM;
    group 3 reads cols 372:500 so all slices stay in-bounds (its valid
    rows sit at partition offset 3)
  - norm: one Sqrt+recip per (sb, c) over the 4 zero-lag columns, then
    relu(acf*rcc) per group split ACT/DVE; channel mean on GpSimd;
    2 store descriptors per sb
  - PE order per iteration: I(s-1,c1) R(s,c0) R(s,c1) I(s,c0) with
    fronts prefetched one iteration ahead -> PSUM fits in 8 banks
    (4 fft + 2x2 acfp)
"""

import sys

import numpy as np

sys.path.insert(0, "/opt/trn_rl_repo")

B, F, T, C = 4, 50, 20000, 2
NUM_FRAME = 300
LEN_FRAME = 512
LAGS = 256
N_CORES = 8
BF_PER_CORE = (B * F) // N_CORES  # 25

FRAMES_PER_SB = 20
N_SB = NUM_FRAME // FRAMES_PER_SB  # 15
NCOLS = 500  # (20 frames x 25 bf) per channel

STARTS = np.linspace(0, T - LEN_FRAME, NUM_FRAME).astype(np.int64)

# irfft group -> (stationary start col, valid-row offset)
GROUP_COL = [0, 125, 250, 372]
GROUP_RO = [0, 0, 0, 3]

# norm relu-scale engine split: groups on ACT (rest on DVE)
F_ACT_GROUPS = (1, 3)


def build_weights():
    t = np.arange(LEN_FRAME, dtype=np.float64)
    w = 0.5 - 0.5 * np.cos(2.0 * np.pi * t / LEN_FRAME)  # periodic hann
    n = np.arange(128, dtype=np.float64)
    b = np.arange(128, dtype=np.float64)
    smat = np.zeros((8, 128, 128))
    for j in range(2):
        ng = n + 128 * j
        ang_e = 2 * np.pi * np.outer(ng, b) / 256.0
        ser = np.cos(ang_e)
        sei = -np.sin(ang_e)
        sei[:, 0] = np.cos(np.pi * ng)  # bin-256 real column
        ang_o = 2 * np.pi * np.outer(ng, 2 * b + 1) / 512.0
        smat[0 + j] = ser
        smat[2 + j] = sei
        smat[4 + j] = np.cos(ang_o)
        smat[6 + j] = -np.sin(ang_o)
    alpha = 0.25  # folds the channel-mean 0.5 (output scales with sqrt(alpha))
    k = np.arange(257.0)
    coef = np.full(257, 2.0)
    coef[0] = 1.0
    coef[256] = 1.0
    D = (alpha / LEN_FRAME) * coef[:, None] * np.cos(
        2 * np.pi * np.outer(k, np.arange(LAGS)) / LEN_FRAME
    )
    dmat = np.zeros((3, 128, 256))
    dmat[0] = D[2 * np.arange(128)]  # bins 0,2,...,254
    dmat[1] = dmat[0]
    dmat[1][0] = D[256]  # ei row 0 carries bin 256 exactly
    dmat[2] = D[2 * np.arange(128) + 1]  # odd bins
    wmat = w.reshape(4, 128).T.copy()  # [128, 4] f32 window columns
    return smat, dmat, wmat


def build_nc(n_sb=N_SB):
    from contextlib import ExitStack

    import concourse.bacc as bacc
    import concourse.tile as tile
    from concourse import mybir

    f32 = mybir.dt.float32
    bf16 = mybir.dt.bfloat16
    AF = mybir.ActivationFunctionType
    ALU = mybir.AluOpType

    nc = bacc.Bacc("TRN2", target_bir_lowering=False, debug=False)

    yt_d = nc.dram_tensor(
        "yt", [N_SB, C, 128, 4, NCOLS], bf16, kind="ExternalInput"
    ).ap()
    smat_d = nc.dram_tensor("smat", [8, 128, 128], bf16, kind="ExternalInput").ap()
    dmat_d = nc.dram_tensor("dmat", [3, 128, 256], bf16, kind="ExternalInput").ap()
    wmat_d = nc.dram_tensor("wmat", [128, 4], f32, kind="ExternalInput").ap()
    out = nc.dram_tensor(
        "out", [BF_PER_CORE, NUM_FRAME, LAGS], f32, kind="ExternalOutput"
    ).ap()

    with tile.TileContext(nc) as tc, ExitStack() as ctx:
        consts = ctx.enter_context(tc.tile_pool(name="consts", bufs=1))
        sbp = ctx.enter_context(tc.tile_pool(name="work", bufs=1))
        pp = ctx.enter_context(tc.tile_pool(name="ps", bufs=1, space="PSUM"))

        yt_q = {}  # (s, c) -> yt tile [128, 4, 500]

        def load_yt(s):
            for c in range(C):
                eng = nc.gpsimd if c == 0 else nc.sync
                t = sbp.tile([128, 4, NCOLS], bf16, tag="yt", bufs=8)
                eng.dma_start(out=t[:], in_=yt_d[s, c])
                yt_q[(s, c)] = t

        load_yt(0)

        # ---- constants ----
        s_sb = [
            consts.tile([128, 128], bf16, tag=f"s{i}", name=f"s_sb{i}")
            for i in range(8)
        ]
        for i in range(8):
            eng = nc.sync if i % 2 == 0 else nc.scalar
            eng.dma_start(out=s_sb[i][:], in_=smat_d[i])
        d_sb = [
            consts.tile([128, 256], bf16, tag=f"d{i}", name=f"d_sb{i}")
            for i in range(3)
        ]
        for i in range(3):
            nc.scalar.dma_start(out=d_sb[i][:], in_=dmat_d[i])
        w_sb = consts.tile([128, 4], f32, tag="w")
        nc.sync.dma_start(out=w_sb[:], in_=wmat_d)
        zero_b = consts.tile([128, 1], f32, tag="zerob")
        nc.vector.memset(zero_b[:], 0.0)
        eps_b = consts.tile([128, 1], f32, tag="epsb")
        nc.vector.memset(eps_b[:], 1e-30)
        zeros_l = consts.tile([128, LAGS], f32, tag="zerosl")
        nc.vector.memset(zeros_l[:], 0.0)

        ad_q = {}  # (s, c) -> (a0, a1, d0, d1)
        sq_q = {}  # (s, c) -> [sq_er, sq_ei, sq_or, sq_oi]
        acf_q = {}  # (s, c) -> acfp psum tile
        nts_q = {}  # (s, c) -> nts tile

        def front(s, c):
            yt = yt_q[(s, c)]
            wy2 = sbp.tile([128, NCOLS], bf16, tag="wy", bufs=4)
            nc.vector.tensor_scalar(
                out=wy2[:], in0=yt[:, 2, :], scalar1=w_sb[:, 2:3], scalar2=None,
                op0=ALU.mult,
            )
            wy3 = sbp.tile([128, NCOLS], bf16, tag="wy", bufs=4)
            nc.vector.tensor_scalar(
                out=wy3[:], in0=yt[:, 3, :], scalar1=w_sb[:, 3:4], scalar2=None,
                op0=ALU.mult,
            )
            ads = []
            for (k, wy, ops) in ((0, wy2, (ALU.add, ALU.subtract)),
                                 (1, wy3, (ALU.add, ALU.subtract))):
                for op1 in ops:
                    tile_ = sbp.tile([128, NCOLS], bf16, tag="ad", bufs=10)
                    nc.vector.scalar_tensor_tensor(
                        out=tile_[:], in0=yt[:, k, :], scalar=w_sb[:, k : k + 1],
                        in1=wy[:], op0=ALU.mult, op1=op1,
                    )
                    ads.append(tile_)
            # ads order: a0, d0, a1, d1
            ad_q[(s, c)] = (ads[0], ads[2], ads[1], ads[3])
            yt_q.pop((s, c), None)

        def R(s, c):
            a0, a1, d0, d1 = ad_q[(s, c)]
            sqs = []
            for (i, m0, m1) in ((0, a0, a1), (2, a0, a1), (4, d0, d1), (6, d0, d1)):
                ps = pp.tile([128, NCOLS], f32, tag="fft", bufs=4)
                nc.tensor.matmul(ps[:], s_sb[i][:], m0[:], start=True, stop=False)
                nc.tensor.matmul(ps[:], s_sb[i + 1][:], m1[:], start=False, stop=True)
                sq = sbp.tile([128, NCOLS], bf16, tag="sq", bufs=10)
                nc.scalar.activation(sq[:], ps[:], AF.Square, bias=zero_b[:])
                sqs.append(sq)
            sq_q[(s, c)] = sqs
            ad_q.pop((s, c), None)

        def I(s, c):
            sq_er, sq_ei, sq_or, sq_oi = sq_q[(s, c)]
            acfp = pp.tile([128, 4, LAGS], f32, tag="acf", bufs=2)
            for g in range(4):
                sl = slice(GROUP_COL[g], GROUP_COL[g] + 128)
                nc.tensor.matmul(
                    acfp[:, g, :], sq_er[:, sl], d_sb[0][:], start=True, stop=False
                )
                nc.tensor.matmul(
                    acfp[:, g, :], sq_ei[:, sl], d_sb[1][:], start=False, stop=False
                )
                nc.tensor.matmul(
                    acfp[:, g, :], sq_or[:, sl], d_sb[2][:], start=False, stop=False
                )
                nc.tensor.matmul(
                    acfp[:, g, :], sq_oi[:, sl], d_sb[2][:], start=False, stop=True
                )
            acf_q[(s, c)] = acfp
            sq_q.pop((s, c), None)

        def norm(s, c):
            acfp = acf_q[(s, c)]
            sqc = sbp.tile([128, 4], f32, tag="sqc", bufs=4)
            nc.scalar.activation(sqc[:], acfp[:, :, 0], AF.Sqrt, bias=eps_b[:])
            rcc = sbp.tile([128, 4], f32, tag="rcc", bufs=4)
            nc.vector.reciprocal(out=rcc[:], in_=sqc[:])
            nts = sbp.tile([128, 4, LAGS], f32, tag=f"nts{c}", bufs=2)
            for g in range(4):
                if g in F_ACT_GROUPS:
                    nc.scalar.activation(
                        nts[:, g, :], acfp[:, g, :], AF.Relu,
                        bias=zero_b[:], scale=rcc[:, g : g + 1],
                    )
                else:
                    nc.vector.scalar_tensor_tensor(
                        out=nts[:, g, :], in0=acfp[:, g, :],
                        scalar=rcc[:, g : g + 1], in1=zeros_l[:, :],
                        op0=ALU.mult, op1=ALU.max,
                    )
            nts_q[(s, c)] = nts
            acf_q.pop((s, c), None)

        def store_sb(s):
            nts0 = nts_q.pop((s, 0))
            nts1 = nts_q.pop((s, 1))
            mt = sbp.tile([128, 4, LAGS], f32, tag="mt", bufs=3)
            nc.gpsimd.tensor_add(mt[:], nts0[:], nts1[:])
            m0 = s * FRAMES_PER_SB
            # per-group stores; group 3 sits at partition offset 3
            for g in range(4):
                ro = GROUP_RO[g]
                mf = m0 + 5 * g
                eng = nc.sync if g % 2 == 0 else nc.gpsimd
                eng.dma_start(
                    out=out[:, mf : mf + 5, :].rearrange("bf mm l -> mm bf l"),
                    in_=mt[ro : ro + 125, g, :],
                )

        # ---- pipeline ----
        load_yt(1)
        front(0, 0)
        load_yt(2)
        front(0, 1)

        for it in range(n_sb):
            s1 = it - 1
            if it + 3 < n_sb:
                load_yt(it + 3)
            if s1 >= 0:
                I(s1, 1)
                norm(s1, 1)
                store_sb(s1)
            R(it, 0)
            if it + 1 < n_sb:
                front(it + 1, 0)
            R(it, 1)
            if it + 1 < n_sb:
                front(it + 1, 1)
            I(it, 0)
            norm(it, 0)

        it = n_sb - 1
        I(it, 1)
        norm(it, 1)
        store_sb(it)

    nc.compile()
    return nc


_NC_CACHE = {}


def _get_nc():
    if "nc" not in _NC_CACHE:
        _NC_CACHE["nc"] = build_nc()
    return _NC_CACHE["nc"]


def make_in_maps(nerv):
    import ml_dtypes

    bf16 = ml_dtypes.bfloat16
    xs = nerv.reshape(B * F, T, C)
    idx = STARTS[:, None] + np.arange(LEN_FRAME)  # [300, 512]
    smat, dmat, wmat = build_weights()
    smat = smat.astype(bf16)
    dmat = dmat.astype(bf16)
    wmat = np.ascontiguousarray(wmat.astype(np.float32))
    maps = []
    for i in range(N_CORES):
        xc = xs[BF_PER_CORE * i : BF_PER_CORE * (i + 1)]  # [25, T, 2]
        fr = xc[:, idx, :].astype(bf16)  # [25, 300, 512, 2]
        # -> [sb, c, t(128), k(4), m_local, bf]
        yt = fr.reshape(BF_PER_CORE, N_SB, FRAMES_PER_SB, 4, 128, C).transpose(
            1, 5, 4, 3, 2, 0
        )
        yt = np.ascontiguousarray(yt).reshape(N_SB, C, 128, 4, NCOLS)
        maps.append({"yt": yt, "smat": smat, "dmat": dmat, "wmat": wmat})
    return maps


def kernel(nervegram, trace=False, **_ignored):
    from concourse.bass_utils import run_bass_kernel_spmd

    nerv = np.ascontiguousarray(np.asarray(nervegram, dtype=np.float32))
    assert nerv.shape == (B, F, T, C)
    in_maps = make_in_maps(nerv)
    nc = _get_nc()
    res = run_bass_kernel_spmd(nc, in_maps, list(range(N_CORES)), trace=trace)
    full = np.concatenate([res.results[i]["out"] for i in range(N_CORES)], axis=0)
    out = full.reshape(B, F, NUM_FRAME, LAGS)
    if trace:
        return out, res
    return out
